# revision 13
# baseline (speedup 1.0000x reference)
"""Causal attention kernel for Trainium2 (8 NeuronCores).

Problem: B=2, H=16, S=2048, D=64 causal attention with a softmax whose
global-max subtraction cancels mathematically (softmax is shift-invariant),
so an unshifted softmax is numerically equivalent.

Sharding: the 32 (b,h) heads are split 4-per-core across 8 cores
(head-parallel, no communication). Q and K are pre-transposed on the host to
[head, D, S] bf16 so the on-chip [d, s] layout (contraction dim d on
partitions) loads with contiguous DMA.

Per-core kernel (per head, scores computed in S^T = [k, q] layout):
  - QK: S^T[k_chunk, q_block] = matmul(lhsT=Kt chunk [64,128],
    rhs=Qt block [64,512]) in bf16, grouped two chunks per PSUM tile
    ([128, 1024], 2 banks).
  - exp(0.125 * S^T): split across THREE engines, load-balanced per unit:
      * ACT: exact table exp (scale folded in), bf16 out.
      * DVE/GpSimd: Schraudolph bit-trick exp in ONE tensor_scalar each:
        e_bf16 = bitcast( int16( s*(0.125*log2e*128) + (16256 - 7.37) ) ).
        The -7.37 centers the piecewise-linear mantissa error so its mean
        ratio vs true exp is 1.0000 (softmax-normalization-safe when mixed
        with exact-ACT columns); residual ~1.7% rms noise on those columns.
    Diagonal 128-blocks of e are triangle-zeroed AFTER the exp by
    affine_select on GpSimd.
  - PV (restructured vs the usual form): e is the STATIONARY operand.
    For each 128-wide q sub-block j and k-chunk ki:
      po[q 128, 65] += matmul(lhsT=e[k 128, q 128], rhs=Vplus[k 128, 65])
    where Vplus carries a ones column so column 64 accumulates softmax row
    sums. Only 65 columns stream per (q,k) block pair (the cost model charges
    matmuls by output free size; ldweights is free), and the output lands
    directly in [q, d] layout - no PE transposes, no PSUM->SBUF copy.
  - Epilogue per q-block: one reciprocal [128,4] of the sum columns, one
    broadcast multiply to f32 SBUF, one DMA per 512 rows.

Scheduling: PV matmuls and epilogues go through a deferred-action FIFO that
trails the QK/exp stream; head 0 loads in chunks on the SP+ACT HWDGE queues
so compute starts early; later heads load whole tensors via GpSimd SWDGE,
emitted one q-block into the previous head so the prefetch has slack; PE
warmup matmuls start the clock ramp at t=0; the last head runs its q-blocks
largest-first so the pipeline drains on the smallest block.
"""

import numpy as np

B, H, S, D = 2, 16, 2048, 64
N_CORES = 8
HPC = (B * H) // N_CORES  # heads per core = 4
QB = 512  # q-block width
KB = 128  # k-chunk width
NQB = S // QB  # 4
NKB = S // KB  # 16

# Schraudolph exp constants: e = bitcast_bf16(int16(s*C1 + C2)) ~ exp(s/8)
C1 = 0.125 * 1.4426950408889634 * 128.0
C2 = 16256.0 - 7.37

# engine-busy cost estimates (ns) for the exp balancer
ACT_NS = 0.8333
DVE_NS = 1.0417
POOL_NS = 1.389

_CACHED = {}


def _build_nc():
    import concourse.bacc as bacc
    import concourse.mybir as mybir
    from concourse.tile import TileContext

    f32 = mybir.dt.float32
    bf16 = mybir.dt.bfloat16
    i16 = mybir.dt.int16
    EXP = mybir.ActivationFunctionType.Exp
    MULT = mybir.AluOpType.mult
    ADD = mybir.AluOpType.add

    nc = bacc.Bacc()
    Qd = nc.declare_dram_parameter("Qt", [HPC, D, S], bf16, isOutput=False)
    Kd = nc.declare_dram_parameter("Kt", [HPC, D, S], bf16, isOutput=False)
    Vd = nc.declare_dram_parameter("V", [HPC, S, D], bf16, isOutput=False)
    Od = nc.declare_dram_parameter("out", [HPC, S, D], f32, isOutput=True)

    with TileContext(nc) as tc:
        with (
            tc.tile_pool(name="consts", bufs=1) as cpool,
            tc.tile_pool(name="qt", bufs=2) as qt_pool,
            tc.tile_pool(name="kt", bufs=2) as kt_pool,
            tc.tile_pool(name="vp", bufs=2) as v_pool,
            tc.tile_pool(name="e", bufs=8) as e_pool,
            tc.tile_pool(name="oo", bufs=3) as oo_pool,
            tc.tile_pool(name="r", bufs=3) as r_pool,
            tc.tile_pool(name="ps", bufs=3, space="PSUM") as ps_pool,
            tc.tile_pool(name="po", bufs=2, space="PSUM") as po_pool,
        ):
            # PE warmup: dummy matmuls so the clock ramp starts at t=0.
            # memsets go on GpSimd, which is otherwise idle at startup.
            wa = cpool.tile([64, 128], bf16)
            wb = cpool.tile([64, 512], bf16)
            nc.gpsimd.memset(wa[:], 0.0)
            nc.gpsimd.memset(wb[:], 0.0)
            wp = ps_pool.tile([KB, 2 * QB], f32, tag="ps")
            for _ in range(6):
                nc.tensor.matmul(wp[:, 0:QB], lhsT=wa[:], rhs=wb[:], start=True, stop=True)

            # exp engine balancer state: projected busy-ns per engine
            busy = {"act": 0.0, "dve": 0.0, "pool": 0.0}

            def do_exp(e_ap_bf16, ps_ap, cols):
                # GPSIMD cannot read PSUM, so only ACT and DVE split the exp
                ca = cols * ACT_NS + 185.0
                cd = cols * DVE_NS + 125.0
                fin = {
                    "act": busy["act"] + ca,
                    "dve": busy["dve"] + cd,
                }
                eng = min(fin, key=fin.get)
                busy[eng] = fin[eng]
                if eng == "act":
                    nc.scalar.activation(e_ap_bf16, ps_ap, EXP, scale=0.125)
                else:
                    nc.vector.tensor_scalar(
                        e_ap_bf16.bitcast(i16), ps_ap, C1, C2, MULT, ADD
                    )

            def causal_zero(e_blk):
                # zero e[i, j] for j < i (future positions) on GpSimd
                busy["pool"] += 273.0
                nc.gpsimd.affine_select(
                    out=e_blk,
                    in_=e_blk,
                    compare_op=mybir.AluOpType.is_ge,
                    fill=0.0,
                    base=0,
                    pattern=[[1, KB]],
                    channel_multiplier=-1,
                )

            def load_head(h):
                qt = qt_pool.tile([D, S], bf16, tag="qt")
                kt = kt_pool.tile([D, S], bf16, tag="kt")
                vp = v_pool.tile([128, NKB, D + 1], bf16, tag="vp")
                nc.gpsimd.memset(vp[:, :, D], 1.0)
                vr = Vd[h].rearrange("(c p) d -> p c d", p=128)
                if h == 0:
                    # prologue: nothing to overlap with - fast HWDGE queues.
                    # qt part 1 is emitted first: the shared HWDGE device
                    # serializes descriptor generation in emission order and
                    # the first QK needs qt+kt part 1 together.
                    parts = [(0, 512), (512, 1024), (1024, 2048)]
                    for pi, (a, b) in enumerate(parts):
                        sl = slice(a, b)
                        nc.scalar.dma_start(out=qt[:, sl], in_=Qd[h, :, sl])
                        nc.sync.dma_start(out=kt[:, sl], in_=Kd[h, :, sl])
                        csl = slice(a // KB, b // KB)
                        nc.sync.dma_start(out=vp[:, csl, 0:D], in_=vr[:, csl, :])
                else:
                    # steady state: SWDGE on GpSimd, off the compute queues
                    nc.gpsimd.dma_start(out=kt[:], in_=Kd[h])
                    nc.gpsimd.dma_start(out=qt[:], in_=Qd[h])
                    nc.gpsimd.dma_start(out=vp[:, :, 0:D], in_=vr[:])
                return qt, kt, vp

            # deferred-action FIFO: PV matmul batches and epilogues trail the
            # QK/exp stream so nothing data-waits at the PE queue head
            actions = []

            def pump(limit=3, depth=3):
                n = 0
                while actions and len(actions) > depth and n < limit:
                    actions.pop(0)()
                    n += 1

            def make_pv(po, vp, qb, pvs):
                # pvs: list of (ki, j, e_slice_ap, start, stop)
                # NOTE: start=True clears has_written for the WHOLE PSUM bank
                # (hw-verified), so exactly one start (the block's first PV)
                # and one stop (its last PV) are allowed per po bank; every
                # other mm relies on per-element has_written=0 -> overwrite
                # for its own first touch of a region.
                def act():
                    for ki, j, e_ap, st, sp in pvs:
                        nc.tensor.matmul(
                            po[:, j, :],
                            lhsT=e_ap,
                            rhs=vp[:, ki, :],
                            start=st,
                            stop=sp,
                            skip_group_check=True,
                        )

                return act

            def make_epilogue(h, qb, po):
                def act():
                    busy["dve"] += 520.0
                    q0 = qb * QB
                    r = r_pool.tile([128, 4], f32, tag="r")
                    nc.vector.reciprocal(r[:], po[:, :, D])
                    oo = oo_pool.tile([128, 4, D], f32, tag="oo")
                    nc.vector.tensor_mul(
                        oo[:],
                        po[:, :, 0:D],
                        r[:].unsqueeze(2).broadcast_to([128, 4, D]),
                    )
                    nc.sync.dma_start(
                        out=Od[h, q0 : q0 + QB, :].rearrange(
                            "(j p) d -> p j d", p=128
                        ),
                        in_=oo[:],
                    )

                return act

            tiles = load_head(0)
            for h in range(HPC):
                qt, kt, vp = tiles

                def emit_unit(h, qb, po, qt, kt, vp, kind, ki0, n):
                    q0 = qb * QB
                    ps = ps_pool.tile([KB, 2 * QB], f32, tag="ps")
                    e = e_pool.tile([KB, 2 * QB], bf16, tag="e")
                    pvs = []
                    if kind == "grp":
                        for i in range(n):
                            ki = ki0 + i
                            nc.tensor.matmul(
                                ps[:, i * QB : (i + 1) * QB],
                                lhsT=kt[:, ki * KB : (ki + 1) * KB],
                                rhs=qt[:, q0 : q0 + QB],
                                start=True,
                                stop=True,
                            )
                        cols = n * QB
                        do_exp(e[:, 0:cols], ps[:, 0:cols], cols)
                        if ki0 + n - 1 == 4 * qb:  # contains the diagonal
                            causal_zero(e[:, (n - 1) * QB : (n - 1) * QB + KB])
                        for i in range(n):
                            ki = ki0 + i
                            for j in range(4):
                                pvs.append(
                                    (
                                        ki,
                                        j,
                                        e[:, i * QB + j * KB : i * QB + (j + 1) * KB],
                                        ki == 0 and j == 0,
                                        False,
                                    )
                                )
                    else:
                        # packed partial-diagonal chunks, bank-aligned:
                        # off=128 -> [0:384], off=384 -> [384:512],
                        # off=256 -> [512:768]
                        segs = []
                        for off, base in ((KB, 0), (3 * KB, 384), (2 * KB, 512)):
                            w = QB - off
                            kk = 4 * qb + off // KB
                            nc.tensor.matmul(
                                ps[:, base : base + w],
                                lhsT=kt[:, kk * KB : (kk + 1) * KB],
                                rhs=qt[:, q0 + off : q0 + QB],
                                start=True,
                                stop=True,
                            )
                            segs.append((kk, off, base, w))
                        do_exp(e[:, 0:768], ps[:, 0:768], 768)
                        for kk, off, base, w in segs:
                            causal_zero(e[:, base : base + KB])
                        for kk, off, base, w in segs:
                            for j in range(off // KB, 4):
                                pvs.append(
                                    (
                                        kk,
                                        j,
                                        e[:, base + j * KB - off : base + j * KB - off + KB],
                                        False,
                                        False,
                                    )
                                )
                        # the packed unit is always the block's last:
                        # its final pv carries the bank's single stop
                        ki_l, j_l, ap_l, _, _ = pvs[-1]
                        pvs[-1] = (ki_l, j_l, ap_l, False, True)
                    actions.append(make_pv(po, vp, qb, pvs))
                    pump()

                def block_thunks(h, qb, qt, kt, vp):
                    # returns unit-emission thunks; the block's epilogue is
                    # appended by the caller after the last thunk runs
                    po = po_pool.tile([128, NQB, D + 1], f32, tag="po")
                    nfull = 4 * qb + 1
                    units = []
                    ki0 = 0
                    while ki0 < nfull:
                        nn = min(2, nfull - ki0)
                        units.append(("grp", ki0, nn))
                        ki0 += nn
                    units.append(("packed", 0, 0))
                    return po, [
                        (lambda kind=kind, k0=k0, nn=nn: emit_unit(
                            h, qb, po, qt, kt, vp, kind, k0, nn
                        ))
                        for kind, k0, nn in units
                    ]

                # Small blocks (qb0/qb1) have too little PE work per unit to
                # hide the exp latency of the 3-deep PSUM rotation, so each
                # head interleaves a big block with a small one; the two po
                # accumulators exactly fill the 2-buffer po pool. Head 0 runs
                # smallest-first to match the incremental part loads; the
                # last head ends on a small packed unit for a fast drain.
                if h == 0:
                    pairs = [(0, 1), (2, 3)]
                elif h == HPC - 1:
                    pairs = [(3, 2), (1, 0)]
                else:
                    pairs = [(3, 0), (2, 1)]
                for pi, (qa, qb_) in enumerate(pairs):
                    poA, TA = block_thunks(h, qa, qt, kt, vp)
                    poB, TB = block_thunks(h, qb_, qt, kt, vp)
                    na, nb = len(TA), len(TB)
                    seq = []
                    for i in range(max(na, nb)):
                        if i < na:
                            seq.append(("A", TA[i], i == na - 1))
                        if i < nb:
                            seq.append(("B", TB[i], i == nb - 1))
                    for which, thunk, is_last in seq:
                        thunk()
                        if is_last:
                            qq = qa if which == "A" else qb_
                            pp = poA if which == "A" else poB
                            actions.append(make_epilogue(h, qq, pp))
                            pump()
                    # prefetch the next head midway through this head
                    if pi == 0 and h + 1 < HPC:
                        next_tiles = load_head(h + 1)
                if h + 1 < HPC:
                    tiles = next_tiles  # noqa: F821

            while actions:
                actions.pop(0)()
    nc.finalize()
    return nc


def _get_nc():
    if "nc" not in _CACHED:
        _CACHED["nc"] = _build_nc()
    return _CACHED["nc"]


def kernel(Q, K, V, mask=None, **_ignored):
    import ml_dtypes
    from concourse.bass_utils import run_bass_kernel_spmd

    nc = _get_nc()
    bf16 = ml_dtypes.bfloat16
    Qr = np.ascontiguousarray(
        np.asarray(Q, dtype=np.float32).reshape(B * H, S, D).transpose(0, 2, 1)
    ).astype(bf16)
    Kr = np.ascontiguousarray(
        np.asarray(K, dtype=np.float32).reshape(B * H, S, D).transpose(0, 2, 1)
    ).astype(bf16)
    Vr = np.asarray(V, dtype=np.float32).reshape(B * H, S, D).astype(bf16)
    in_maps = [
        {
            "Qt": Qr[i * HPC : (i + 1) * HPC],
            "Kt": Kr[i * HPC : (i + 1) * HPC],
            "V": Vr[i * HPC : (i + 1) * HPC],
        }
        for i in range(N_CORES)
    ]
    res = run_bass_kernel_spmd(nc, in_maps, core_ids=list(range(N_CORES)))
    out = np.concatenate([res.results[i]["out"] for i in range(N_CORES)], axis=0)
    return out.reshape(B, H, S, D).astype(np.float32)
